# revision 4
# baseline (speedup 1.0000x reference)
"""Trainium2 Bass kernel for MultiHeadCrossAttention (B=8,N=8,Q=128,K=1024,D=512,H=8).

Sharding: data-parallel over batch B — core i handles batch i.
Per-core dataflow (all matmuls bf16 on TensorE, f32 PSUM accumulate):
  - host stages transposed bf16 activations (kvT [D, N*K], qT [D, N*Q]) and
    transposed bf16 weights; SCALE folded into Wq/bq; Gaussian distance bias +
    key mask folded into a precomputed multiplicative exp-bias table [Q, K].
  - Q-proj once up front -> qTp [j, m] (transposed layout, heads on partitions)
  - per step n: K-proj -> kT [j, k] (transposed), V-proj -> v [k, j] (natural)
  - per head: logits = qT'^T @ kT (PSUM, natural [q, k]); exp on ScalarE;
    multiply by exp-bias with fused row-sum accumulation on VectorE;
    DMA-xbar transpose attn -> [k, q]; AV accumulated over k-chunks;
    renormalize by 1/rowsum during PSUM evacuation.
  - out-proj: DMA-transpose attn output, 4 accumulating matmuls, bias added
    during final evacuation; store f32.
"""

import numpy as np
import ml_dtypes

B, N, Q, K, D, H = 8, 8, 128, 1024, 512, 8
HD = D // H
SCALE = HD ** -0.5
SIGMA2 = max(0.35 * 0.35, 1e-6)
NCORES = 8

_BF16 = ml_dtypes.bfloat16

_CACHE = {}


def _build_program():
    import concourse.bass as bass
    import concourse.mybir as mybir
    import concourse.tile as tile
    from concourse import bacc

    f32 = mybir.dt.float32
    bf16 = mybir.dt.bfloat16
    AF = mybir.ActivationFunctionType
    ALU = mybir.AluOpType

    nc = bacc.Bacc("TRN2", target_bir_lowering=False, debug=False,
                   num_devices=NCORES)

    kvT_h = nc.declare_dram_parameter("kvT", [D, N * K], bf16, isOutput=False)
    qT_h = nc.declare_dram_parameter("qT", [D, N * Q], bf16, isOutput=False)
    wq_h = nc.declare_dram_parameter("wqT", [D, D], bf16, isOutput=False)
    wk_h = nc.declare_dram_parameter("wkT", [D, D], bf16, isOutput=False)
    wv_h = nc.declare_dram_parameter("wvT", [D, D], bf16, isOutput=False)
    wo_h = nc.declare_dram_parameter("woT", [D, D], bf16, isOutput=False)
    bq_h = nc.declare_dram_parameter("bq2", [128, 4], f32, isOutput=False)
    bk_h = nc.declare_dram_parameter("bk2", [128, 4], f32, isOutput=False)
    bv_h = nc.declare_dram_parameter("bvb", [128, D], f32, isOutput=False)
    bo_h = nc.declare_dram_parameter("bob", [128, D], f32, isOutput=False)
    eb_h = nc.declare_dram_parameter("expb", [Q, K], bf16, isOutput=False)
    out_h = nc.declare_dram_parameter("out", [N, Q, D], f32, isOutput=True)

    kvT = kvT_h.ap().rearrange("(c p) m -> p c m", p=128)   # [128, 4, N*K]
    qT = qT_h.ap().rearrange("(c p) m -> p c m", p=128)     # [128, 4, N*Q]
    w_aps = {k: h.ap().rearrange("(c p) j -> p c j", p=128)
             for k, h in (("wq", wq_h), ("wk", wk_h), ("wv", wv_h), ("wo", wo_h))}
    out_ap = out_h.ap()

    with tile.TileContext(nc) as tc:
        with (
            tc.tile_pool(name="const", bufs=1) as cpool,
            tc.tile_pool(name="kvin", bufs=2) as kvpool,
            tc.tile_pool(name="kt", bufs=2) as ktpool,
            tc.tile_pool(name="vt", bufs=2) as vtpool,
            tc.tile_pool(name="attn", bufs=3) as apool,
            tc.tile_pool(name="small", bufs=6) as spool,
            tc.tile_pool(name="oav", bufs=2) as opool,
            tc.tile_pool(name="pp", bufs=2, space="PSUM") as pp,
            tc.tile_pool(name="pl", bufs=2, space="PSUM") as pl,
            tc.tile_pool(name="pav", bufs=2, space="PSUM") as pav,
        ):
            # ---- constants ----
            w = {}
            for name in ("wq", "wk", "wv", "wo"):
                w[name] = cpool.tile([128, 4, D], bf16, tag=name, name=name)
                nc.gpsimd.dma_start(out=w[name][:], in_=w_aps[name][:])
            bq2 = cpool.tile([128, 4], f32, tag="bq2", name="bq2")
            nc.gpsimd.dma_start(out=bq2[:], in_=bq_h.ap()[:])
            bk2 = cpool.tile([128, 4], f32, tag="bk2", name="bk2")
            nc.gpsimd.dma_start(out=bk2[:], in_=bk_h.ap()[:])
            bvb = cpool.tile([128, D], f32, tag="bvb", name="bvb")
            nc.gpsimd.dma_start(out=bvb[:], in_=bv_h.ap()[:])
            bob = cpool.tile([128, D], f32, tag="bob", name="bob")
            nc.gpsimd.dma_start(out=bob[:], in_=bo_h.ap()[:])
            expb = cpool.tile([Q, K], bf16, tag="expb", name="expb")
            nc.gpsimd.dma_start(out=expb[:], in_=eb_h.ap()[:])

            # ---- Q projection for all steps: qTp[p, jc, m] = q[m, jc*128+p]
            qin = cpool.tile([128, 4, N * Q], bf16, tag="qin", name="qin")
            nc.gpsimd.dma_start(out=qin[:], in_=qT[:])
            qTp = cpool.tile([128, 4, N * Q], bf16, tag="qTp", name="qTp")
            for jc in range(4):
                for mb in range(2):
                    ps = pp.tile([128, 512], f32, tag="pp", name="pp")
                    for ic in range(4):
                        nc.tensor.matmul(
                            ps[:],
                            w["wq"][:, ic, jc * 128:(jc + 1) * 128],
                            qin[:, ic, mb * 512:(mb + 1) * 512],
                            start=(ic == 0), stop=(ic == 3),
                        )
                    nc.vector.tensor_scalar(
                        out=qTp[:, jc, mb * 512:(mb + 1) * 512], in0=ps[:],
                        scalar1=bq2[:, jc:jc + 1], scalar2=None, op0=ALU.add)

            for n in range(N):
                # ---- load kv^T slice for this step ----
                kvin = kvpool.tile([128, 4, K], bf16, tag="kvin", name="kvin")
                nc.gpsimd.dma_start(
                    out=kvin[:], in_=kvT[:, :, n * K:(n + 1) * K])

                # ---- K projection -> kT[p, jc, k] = k[k, jc*128+p] ----
                kt = ktpool.tile([128, 4, K], bf16, tag="kt", name="kt")
                for jc in range(4):
                    for mb in range(2):
                        ps = pp.tile([128, 512], f32, tag="pp", name="pp")
                        for ic in range(4):
                            nc.tensor.matmul(
                                ps[:],
                                w["wk"][:, ic, jc * 128:(jc + 1) * 128],
                                kvin[:, ic, mb * 512:(mb + 1) * 512],
                                start=(ic == 0), stop=(ic == 3),
                            )
                        nc.vector.tensor_scalar(
                            out=kt[:, jc, mb * 512:(mb + 1) * 512], in0=ps[:],
                            scalar1=bk2[:, jc:jc + 1], scalar2=None, op0=ALU.add)

                # ---- V projection -> vt[p, c, j] = v[c*128+p, j] ----
                vt = vtpool.tile([128, 8, D], bf16, tag="vt", name="vt")
                for mc in range(8):
                    ps = pp.tile([128, 512], f32, tag="pp", name="pp")
                    for ic in range(4):
                        nc.tensor.matmul(
                            ps[:],
                            kvin[:, ic, mc * 128:(mc + 1) * 128],
                            w["wv"][:, ic, :],
                            start=(ic == 0), stop=(ic == 3),
                        )
                    nc.vector.scalar_tensor_tensor(
                        out=vt[:, mc, :], in0=ps[:], scalar=1.0, in1=bvb[:],
                        op0=ALU.mult, op1=ALU.add)

                oav = opool.tile([Q, D], bf16, tag="oav", name="oav")
                for h in range(H):
                    jc, e = h // 2, (h % 2) * 64
                    # ---- logits[q, k] ----
                    psl = pl.tile([Q, K], f32, tag="pl", name="pl")
                    lhs_q = qTp[e:e + 64, jc, n * Q:(n + 1) * Q]
                    for kb in range(2):
                        nc.tensor.matmul(
                            psl[:, kb * 512:(kb + 1) * 512],
                            lhs_q,
                            kt[e:e + 64, jc, kb * 512:(kb + 1) * 512],
                            start=True, stop=True,
                        )
                    # ---- softmax (no max-subtraction; logits are O(1)) ----
                    ae = apool.tile([Q, K], bf16, tag="ae", name="ae")
                    nc.scalar.activation(out=ae[:], in_=psl[:], func=AF.Exp)
                    ab = apool.tile([Q, K], bf16, tag="ab", name="ab")
                    sums = spool.tile([Q, 1], f32, tag="sums", name="sums")
                    nc.vector.scalar_tensor_tensor(
                        out=ab[:], in0=ae[:], scalar=1.0, in1=expb[:],
                        op0=ALU.mult, op1=ALU.mult, accum_out=sums[:])
                    rec = spool.tile([Q, 1], f32, tag="rec", name="rec")
                    nc.vector.reciprocal(rec[:], sums[:])
                    # ---- transpose attn -> [k-part, q] chunks ----
                    abT = apool.tile([128, 8, Q], bf16, tag="abT", name="abT")
                    nc.sync.dma_start_transpose(abT[:], ab[:])
                    # ---- AV: psav[q, hd] ----
                    psav = pav.tile([Q, HD], f32, tag="pav", name="pav")
                    for c in range(8):
                        nc.tensor.matmul(
                            psav[:],
                            abT[:, c, :],
                            vt[:, c, h * HD:(h + 1) * HD],
                            start=(c == 0), stop=(c == 7),
                        )
                    nc.vector.tensor_scalar_mul(
                        out=oav[:, h * HD:(h + 1) * HD], in0=psav[:],
                        scalar1=rec[:])

                # ---- out projection ----
                oavT = opool.tile([128, 4, Q], bf16, tag="oavT", name="oavT")
                nc.sync.dma_start_transpose(oavT[:], oav[:])
                pso = pp.tile([Q, D], f32, tag="pp", name="pp")
                for jc in range(4):
                    nc.tensor.matmul(
                        pso[:], oavT[:, jc, :], w["wo"][:, jc, :],
                        start=(jc == 0), stop=(jc == 3))
                osb = opool.tile([Q, D], f32, tag="osb", name="osb")
                nc.vector.scalar_tensor_tensor(
                    out=osb[:], in0=pso[:], scalar=1.0, in1=bob[:],
                    op0=ALU.mult, op1=ALU.add)
                nc.gpsimd.dma_start(out=out_ap[n], in_=osb[:])

    nc.compile()
    return nc


def _stage_inputs(inputs):
    """Build per-core input maps (host-side sharding + layout)."""
    query = np.asarray(inputs["query"], np.float32)
    key_value = np.asarray(inputs["key_value"], np.float32)
    query_pos = np.asarray(inputs["query_pos"], np.float32)
    key_pos = np.asarray(inputs["key_pos"], np.float32)
    key_mask = np.asarray(inputs["key_mask"])

    wqT = np.ascontiguousarray((np.asarray(inputs["Wq"], np.float32) * SCALE).T
                               ).astype(_BF16)
    wkT = np.ascontiguousarray(np.asarray(inputs["Wk"], np.float32).T).astype(_BF16)
    wvT = np.ascontiguousarray(np.asarray(inputs["Wv"], np.float32).T).astype(_BF16)
    woT = np.ascontiguousarray(np.asarray(inputs["Wo"], np.float32).T).astype(_BF16)
    bq2 = np.ascontiguousarray(
        (np.asarray(inputs["bq"], np.float32) * SCALE).reshape(4, 128).T)
    bk2 = np.ascontiguousarray(np.asarray(inputs["bk"], np.float32).reshape(4, 128).T)
    bvb = np.ascontiguousarray(
        np.broadcast_to(np.asarray(inputs["bv"], np.float32), (128, D)))
    bob = np.ascontiguousarray(
        np.broadcast_to(np.asarray(inputs["bo"], np.float32), (128, D)))

    in_maps = []
    for b in range(B):
        kvT = np.ascontiguousarray(
            key_value[b].reshape(N * K, D).T).astype(_BF16)
        qT = np.ascontiguousarray(query[b].reshape(N * Q, D).T).astype(_BF16)
        d2 = ((query_pos[b][:, None, :] - key_pos[b][None, :, :]) ** 2).sum(-1)
        eb = np.where(key_mask[b][None, :],
                      np.exp(-d2 / (2.0 * SIGMA2)), 0.0).astype(_BF16)
        in_maps.append({
            "kvT": kvT, "qT": qT,
            "wqT": wqT, "wkT": wkT, "wvT": wvT, "woT": woT,
            "bq2": bq2, "bk2": bk2, "bvb": bvb, "bob": bob,
            "expb": eb,
        })
    return in_maps


def _get_runner():
    """Compile (once) and return a callable in_maps -> list of out arrays."""
    if "runner" in _CACHE:
        return _CACHE["runner"]

    import jax
    import jax.numpy as jnp
    from jax.sharding import Mesh, PartitionSpec
    from jax.experimental.shard_map import shard_map
    from concourse import bass2jax
    from concourse.bass2jax import (_bass_exec_p, install_neuronx_cc_hook,
                                    partition_id_tensor)
    import concourse.mybir as mybir

    nc = _build_program()
    install_neuronx_cc_hook()

    in_names = ["kvT", "qT", "wqT", "wkT", "wvT", "woT",
                "bq2", "bk2", "bvb", "bob", "expb"]
    out_shape = (N, Q, D)
    out_aval = jax.core.ShapedArray(out_shape, np.float32)
    all_names = in_names + ["out", "partition_id"]

    def _body(*args):
        outs = _bass_exec_p.bind(
            *args, partition_id_tensor(),
            out_avals=(out_aval,),
            in_names=tuple(all_names),
            out_names=("out",),
            lowering_input_output_aliases=(),
            sim_require_finite=True,
            sim_require_nnan=True,
            nc=nc,
        )
        return tuple(outs)

    n_in = len(in_names)
    devices = jax.devices()[:NCORES]
    mesh = Mesh(np.asarray(devices), ("core",))
    sharded = jax.jit(
        shard_map(_body, mesh=mesh,
                  in_specs=(PartitionSpec("core"),) * (n_in + 1),
                  out_specs=(PartitionSpec("core"),),
                  check_rep=False),
        donate_argnums=(n_in,), keep_unused=True)

    def runner(in_maps):
        concat_in = [np.concatenate([np.asarray(m[name]) for m in in_maps], axis=0)
                     for name in in_names]
        zeros = np.zeros((NCORES * N, Q, D), np.float32)
        (out,) = sharded(*concat_in, zeros)
        out = np.asarray(out).reshape(NCORES, N, Q, D)
        return out

    _CACHE["runner"] = runner
    _CACHE["sharded"] = sharded
    _CACHE["mesh"] = mesh
    _CACHE["in_names"] = in_names
    _CACHE["nc"] = nc
    return runner


def kernel(**inputs):
    runner = _get_runner()
    in_maps = _stage_inputs(inputs)
    out = runner(in_maps)          # [8 cores = B, N, Q, D]
    return np.ascontiguousarray(out)
